# revision 30
# baseline (speedup 1.0000x reference)
"""Trainium2 Bass kernel for the MultiHeadBDH fast-weight layer.

Math: the reference is a sequential Hebbian fast-weight scan
    y_t = w_t @ x_t ;  w_{t+1} = c_t * w_t + d_t * x_t x_t^T
with per-(b,h,t) scalars c_t = m_t*alpha_h + (1-m_t), d_t = m_t*eta_h.
Since the decay is scalar and the update rank-1, the scan has an exact
closed form (linear-attention style), all matmuls:

    cm[t]  = inclusive cumsum of mask, cmx[t] = cm[t]-m[t], cmT = cm[T-1]
    P_h[t] = exp(la_h*cmx[t])                       (la_h = log alpha_h)
    E_h[u,t] = exp(la_h*((cmx[t]-cm[u]) + 1e9*[t<=u]))
    G[u,t] = sum_n X[u,n]X[t,n]                     (Gram matrix)
    Y[b,:,h,:]  = P_h . (X @ w0^T) + (E_h*G)^T @ ((m*eta_h) . X)
    wf[b,h] = exp(la_h*cmT)*w0 + ((ec_h*m*eta_h) . X)^T @ X
    ec_h[u] = exp(la_h*(cmT-cm[u]))

Sharding: pure data parallel, B=16 batches over 8 cores (2 per core).
"""

import numpy as np

import concourse.bass as bass
import concourse.bacc as bacc
import concourse.mybir as mybir
import concourse.tile as tile
from concourse.bass_utils import run_bass_kernel_spmd
from concourse.masks import make_identity

B, T, N, H = 16, 128, 256, 4
NCORES = 8
BPC = B // NCORES  # batches per core

F32 = mybir.dt.float32
F32R = mybir.dt.float32r
I32 = mybir.dt.int32
AF = mybir.ActivationFunctionType
OP = mybir.AluOpType

EW = 131  # per-head exp-block width: 128 (E cols) + P + LT + ec


def build_nc():
    nc = bacc.Bacc(None)

    x_d = nc.dram_tensor("x_sh", [BPC, T, N], F32R, kind="ExternalInput")
    w0_d = nc.dram_tensor("w0_sh", [BPC, N, N], F32R, kind="ExternalInput")
    mk_d = nc.dram_tensor("mask_sh", [BPC, T], I32, kind="ExternalInput")
    le_d = nc.dram_tensor("laeta", [1, 2 * H], F32, kind="ExternalInput")
    wf_d = nc.dram_tensor("wf_out", [BPC, H, N, N], F32, kind="ExternalOutput")
    y_d = nc.dram_tensor("y_out", [BPC, T, H, N], F32, kind="ExternalOutput")

    with tile.TileContext(nc) as tc:
        with (
            tc.tile_pool(name="const", bufs=1) as cpool,
            tc.tile_pool(name="core", bufs=1) as kpool,
            tc.tile_pool(name="batch", bufs=2) as bpool,
            tc.tile_pool(name="ps_a", bufs=2, space="PSUM") as ps_a,
            tc.tile_pool(name="ps_g", bufs=2, space="PSUM") as ps_g,
            tc.tile_pool(name="ps_y2", bufs=2, space="PSUM") as ps_y2,
            tc.tile_pool(name="ps_wf", bufs=2, space="PSUM") as ps_wf,
        ):
            # ---------------- constants ----------------
            ident = cpool.tile([128, 128], F32)
            make_identity(nc, ident[:])
            identr = cpool.tile([128, 128], F32R)
            nc.vector.tensor_copy(identr[:], ident[:])
            # MK[u,t] = 1e9 where t<=u else 0  (pre-exp causal mask)
            mk = cpool.tile([128, 128], F32)
            nc.gpsimd.memset(mk[:], 0.0)
            # keep 0 where t-u-1 >= 0 (u < t), fill 1e9 where t <= u
            nc.gpsimd.affine_select(
                out=mk[:], in_=mk[:], compare_op=OP.is_ge, fill=1e9,
                base=-1, pattern=[[1, 128]], channel_multiplier=-1,
            )
            # ones row for K=1 broadcast matmuls (replicate a row across partitions)
            ones1 = cpool.tile([1, 128], F32)
            nc.gpsimd.memset(ones1[:], 1.0)
            # la/eta broadcast to all partitions: cols 0..3 = log(alpha), 4..7 = eta
            le_row = cpool.tile([1, 2 * H], F32)
            nc.gpsimd.dma_start(le_row[:], le_d[:])
            pe8_ps = ps_a.tile([128, 2 * H], F32, tag="a")
            nc.tensor.matmul(pe8_ps[:], ones1[:], le_row[:], start=True, stop=True)
            pe8 = cpool.tile([128, 2 * H], F32)
            nc.scalar.copy(pe8[:], pe8_ps[:])

            # ---------------- whole-core loads ----------------
            # x/w0 on the Sync HWDGE ring (issued first: they gate PE);
            # mask on the gpsimd SWDGE ring in parallel.
            X2 = kpool.tile([128, BPC * N], F32R)  # [t, (b n)]
            nc.sync.dma_start(X2[:].rearrange("p (b n) -> p b n", b=BPC),
                              x_d[:].rearrange("b t n -> t b n"))
            w0sb = kpool.tile([128, BPC * 2 * N], F32R)  # [p, (b c m)]
            nc.sync.dma_start(w0sb[:].rearrange("p (b c m) -> p b c m", b=BPC, c=2),
                              w0_d[:].rearrange("b (c p) m -> p b c m", p=128))
            mi = kpool.tile([1, BPC * T], I32)
            for b in range(BPC):
                nc.gpsimd.dma_start(mi[0:1, b * T:(b + 1) * T], mk_d[b:b + 1, :])

            # ---------------- mask scalar chain (all on partition 0) ----------------
            # rows segments per batch: [b*384+0:128]=mask(f32), [+128]=cm, [+256]=cmx
            rows = kpool.tile([1, BPC * 3 * T], F32)
            for b in range(BPC):
                o = b * 3 * T
                nc.gpsimd.tensor_copy(rows[0:1, o:o + T], mi[0:1, b * T:(b + 1) * T])
                nc.vector.tensor_tensor_scan(
                    rows[0:1, o + T:o + 2 * T], rows[0:1, o:o + T],
                    rows[0:1, o:o + T], 0.0, OP.add, OP.bypass)
                nc.vector.tensor_tensor(
                    rows[0:1, o + 2 * T:o + 3 * T], rows[0:1, o + T:o + 2 * T],
                    rows[0:1, o:o + T], OP.subtract)

            # both batches share one exp-input tile: cols 0..127 = OD(+mask),
            # 128..130 = [cmx, cmT, ccol]; batch b at offset b*EW
            ODx2 = kpool.tile([128, BPC * EW], F32)
            E42 = kpool.tile([128, H * BPC * EW], F32)

            for b in range(BPC):
                xb = X2[:, b * N:(b + 1) * N]
                o = b * 3 * T

                # columns from partition-0 rows via K=1 matmuls:
                # cols_b[:,0]=cm, 1=cmx, 2=m (=cm-cmx), 3=cmT (from bcc col 0)
                cols_ps = ps_a.tile([128, 2], F32, tag="a")
                for j in range(2):
                    nc.tensor.matmul(cols_ps[:, j:j + 1],
                                     rows[0:1, o + (j + 1) * T:o + (j + 2) * T],
                                     ones1[0:1, 0:1], start=True, stop=True)
                cols_b = bpool.tile([128, 4], F32)
                nc.scalar.copy(cols_b[:, 0:2], cols_ps[:])

                # ---------------- transposes ----------------
                xt_ps = ps_a.tile([128, 256], F32R, tag="a")
                for d in range(2):
                    nc.tensor.transpose(
                        xt_ps[:, d * 128:(d + 1) * 128],
                        X2[:, b * N + d * 128: b * N + (d + 1) * 128], identr[:])
                XTsb = bpool.tile([128, 256], F32R)
                nc.scalar.copy(XTsb[:], xt_ps[:])

                w0t_ps = ps_a.tile([128, 512], F32R, tag="a")
                for c in range(2):
                    for d in range(2):
                        nc.tensor.transpose(
                            w0t_ps[:, d * 256 + c * 128: d * 256 + (c + 1) * 128],
                            w0sb[:, (b * 2 + c) * N + d * 128: (b * 2 + c) * N + (d + 1) * 128],
                            identr[:])
                w0Tsb = bpool.tile([128, 512], F32R)
                nc.scalar.copy(w0Tsb[:], w0t_ps[:])

                # ---------------- G = X X^T (PSUM, [u,t]) ----------------
                g_ps = ps_g.tile([128, 128], F32, tag="g")
                for d in range(2):
                    nc.tensor.matmul(
                        g_ps[:], XTsb[:, d * 128:(d + 1) * 128],
                        XTsb[:, d * 128:(d + 1) * 128],
                        start=(d == 0), stop=(d == 1))

                # ---------------- Y1 = X @ w0^T ----------------
                y1_ps = ps_a.tile([128, 256], F32, tag="a")
                for d in range(2):
                    nc.tensor.matmul(
                        y1_ps[:], XTsb[:, d * 128:(d + 1) * 128],
                        w0Tsb[:, d * 256:(d + 1) * 256],
                        start=(d == 0), stop=(d == 1))
                Y1sb = bpool.tile([128, 256], F32)
                nc.scalar.copy(Y1sb[:], y1_ps[:])

                # ---------------- decay matrix + exponentials ----------------
                # ODx cols 0..127: (cmx[t] - cm[u]) + MK ; col 128: cmx ; 129: cmT ; 130: ccol
                bcc_ps = ps_g.tile([128, 129], F32, tag="g")
                nc.tensor.matmul(bcc_ps[:], ones1[:],
                                 rows[0:1, o + 2 * T - 1:o + 3 * T],
                                 start=True, stop=True)
                nc.scalar.copy(cols_b[:, 2:3], bcc_ps[:, 0:1])
                nc.vector.scalar_tensor_tensor(
                    ODx2[:, b * EW:b * EW + 128], bcc_ps[:, 1:129],
                    cols_b[:, 0:1], mk[:], OP.subtract, OP.add)
                nc.vector.tensor_tensor(cols_b[:, 3:4], cols_b[:, 2:3],
                                        cols_b[:, 0:1], OP.subtract)
                nc.vector.tensor_copy(ODx2[:, b * EW + 128:b * EW + 131],
                                      cols_b[:, 1:4])

                # ---------------- per-head column vectors ----------------
                mEta4 = bpool.tile([128, H], F32)
                nc.vector.scalar_tensor_tensor(
                    mEta4[:], cols_b[:, 0:1].broadcast_to([128, H]),
                    cols_b[:, 1:2], pe8[:, H:2 * H], OP.subtract, OP.mult)

                # exps for this batch's heads (the fused-both-batch variant
                # would serialize on the other batch; per-batch over [128,EW])
                for h in range(H):
                    nc.scalar.activation(
                        E42[:, (h * BPC + b) * EW:(h * BPC + b + 1) * EW],
                        ODx2[:, b * EW:(b + 1) * EW], AF.Exp,
                        scale=pe8[:, h:h + 1])
                eb = lambda h: (h * BPC + b) * EW

                # coef4 = ec * mEta  (ec strided out of E42)
                coef4 = bpool.tile([128, H], F32)
                nc.vector.tensor_tensor(
                    coef4[:].rearrange("p (h c) -> p h c", c=1),
                    E42[:].rearrange("p (h c) -> p h c", c=BPC * EW)[:, :, b * EW + 130:b * EW + 131],
                    mEta4[:].rearrange("p (h c) -> p h c", c=1), OP.mult)

                # AgT_h = (G * mEta_h) * E_h   (folds the m*eta row-scale)
                AgT4 = bpool.tile([128, H * 128], F32R)
                for h in range(H):
                    nc.vector.scalar_tensor_tensor(
                        AgT4[:, h * 128:(h + 1) * 128], g_ps[:],
                        mEta4[:, h:h + 1], E42[:, eb(h):eb(h) + 128],
                        OP.mult, OP.mult)

                # Xc_h = coef_h . X   (on ACT: per-partition scale)
                Xc4 = bpool.tile([128, H * N], F32R)
                for h in range(H):
                    nc.scalar.mul(Xc4[:, h * N:(h + 1) * N], xb,
                                  coef4[:, h:h + 1])

                # ---------------- per-head output blocks ----------------
                Ysb = bpool.tile([128, H * N], F32)
                wfsb = bpool.tile([128, H * 2 * N], F32)
                for h in range(H):
                    # Y2 = AgT_h^T @ X   -> PSUM
                    y2_ps = ps_y2.tile([128, 256], F32, tag="y2")
                    nc.tensor.matmul(
                        y2_ps[:], AgT4[:, h * 128:(h + 1) * 128],
                        xb, start=True, stop=True)
                    # Y = Y1*P + Y2
                    nc.vector.scalar_tensor_tensor(
                        Ysb[:, h * N:(h + 1) * N], Y1sb[:],
                        E42[:, eb(h) + 128:eb(h) + 129], y2_ps[:],
                        OP.mult, OP.add)

                    # wf = LT*w0 + Xc^T X : Sigma on PE, w0-term fused into
                    # the PSUM->SBUF move via scalar_tensor_tensor
                    wf_ps = ps_wf.tile([128, 512], F32, tag="wf")
                    for c in range(2):
                        nc.tensor.matmul(
                            wf_ps[:, c * 256:(c + 1) * 256],
                            Xc4[:, h * N + c * 128: h * N + (c + 1) * 128],
                            xb, start=True, stop=True)
                    nc.vector.scalar_tensor_tensor(
                        wfsb[:, h * 512:(h + 1) * 512],
                        w0sb[:, (b * 2) * N:(b * 2 + 2) * N],
                        E42[:, eb(h) + 129:eb(h) + 130], wf_ps[:],
                        OP.mult, OP.add)

                nc.gpsimd.dma_start(
                    y_d[:].rearrange("b t h n -> b t h n")[b],
                    Ysb[:].rearrange("p (h n) -> p h n", h=H))
                wf_view = wf_d[:].rearrange("b h (c p) m -> b p h c m", p=128)[b]
                for hp in range(2):
                    nc.sync.dma_start(
                        wf_view[:, hp * 2:(hp + 1) * 2],
                        wfsb[:, hp * 1024:(hp + 1) * 1024].rearrange(
                            "p (h c m) -> p h c m", h=2, c=2))

    nc.finalize()
    return nc


_NC_CACHE = None


def _get_nc():
    global _NC_CACHE
    if _NC_CACHE is None:
        _NC_CACHE = build_nc()
    return _NC_CACHE


def make_in_maps(x, w_init, mask, alpha, eta):
    la = np.minimum(
        np.log(np.maximum(np.asarray(alpha, np.float64), 1e-30)), -1e-7
    ).astype(np.float32)
    laeta = np.concatenate([la, np.asarray(eta, np.float32)]).reshape(1, 2 * H)
    maps = []
    for c in range(NCORES):
        sl = slice(BPC * c, BPC * (c + 1))
        maps.append({
            "x_sh": np.ascontiguousarray(np.asarray(x, np.float32)[sl]),
            "w0_sh": np.ascontiguousarray(np.asarray(w_init, np.float32)[sl]),
            "mask_sh": np.ascontiguousarray(np.asarray(mask, np.int32)[sl]),
            "laeta": laeta,
        })
    return maps


def kernel(x, w_init, alpha, eta, mask):
    nc = _get_nc()
    in_maps = make_in_maps(x, w_init, mask, alpha, eta)
    res = run_bass_kernel_spmd(nc, in_maps, list(range(NCORES)))
    wf = np.concatenate([m["wf_out"] for m in res.results], axis=0)
    y = np.concatenate([m["y_out"] for m in res.results], axis=0)
    return wf, y


# revision 31
# speedup vs baseline: 1.1363x; 1.1363x over previous
"""Trainium2 Bass kernel for the MultiHeadBDH fast-weight layer.

Math: the reference is a sequential Hebbian fast-weight scan
    y_t = w_t @ x_t ;  w_{t+1} = c_t * w_t + d_t * x_t x_t^T
with per-(b,h,t) scalars c_t = m_t*alpha_h + (1-m_t), d_t = m_t*eta_h.
Since the decay is scalar and the update rank-1, the scan has an exact
closed form (linear-attention style), all matmuls:

    cm[t]  = inclusive cumsum of mask, cmx[t] = cm[t]-m[t], cmT = cm[T-1]
    P_h[t] = exp(la_h*cmx[t])                       (la_h = log alpha_h)
    E_h[u,t] = exp(la_h*((cmx[t]-cm[u]) + 1e9*[t<=u]))
    G[u,t] = sum_n X[u,n]X[t,n]                     (Gram matrix)
    Y[b,:,h,:]  = P_h . (X @ w0^T) + (E_h*G*m*eta_h)^T @ X
    wf[b,h] = exp(la_h*cmT)*w0 + ((ec_h*m*eta_h) . X)^T @ X
    ec_h[u] = exp(la_h*(cmT-cm[u]))

Sharding: pure data parallel, B=16 batches over 8 cores (2 per core).
"""

import numpy as np

import concourse.bacc as bacc
import concourse.mybir as mybir
import concourse.tile as tile
from concourse.bass_utils import run_bass_kernel_spmd

B, T, N, H = 16, 128, 256, 4
NCORES = 8
BPC = B // NCORES  # batches per core

F32 = mybir.dt.float32
F32R = mybir.dt.float32r
I32 = mybir.dt.int32
AF = mybir.ActivationFunctionType
OP = mybir.AluOpType

EW = 131  # per-(b,h) exp-block width: 128 (E cols) + P + LT + ec


def build_nc():
    nc = bacc.Bacc(None)

    x_d = nc.dram_tensor("x_sh", [BPC, T, N], F32R, kind="ExternalInput")
    w0_d = nc.dram_tensor("w0_sh", [BPC, N, N], F32R, kind="ExternalInput")
    mk_d = nc.dram_tensor("mask_sh", [BPC, T], I32, kind="ExternalInput")
    le_d = nc.dram_tensor("laeta", [1, 2 * H], F32, kind="ExternalInput")
    cir_d = nc.dram_tensor("c_identr", [128, 128], F32R, kind="ExternalInput")
    cmo_d = nc.dram_tensor("c_mko", [128, 256], F32, kind="ExternalInput")
    wf_d = nc.dram_tensor("wf_out", [BPC, H, N, N], F32, kind="ExternalOutput")
    y_d = nc.dram_tensor("y_out", [BPC, T, H, N], F32, kind="ExternalOutput")

    with tile.TileContext(nc) as tc:
        with (
            tc.tile_pool(name="const", bufs=1) as cpool,
            tc.tile_pool(name="core", bufs=1) as kpool,
            tc.tile_pool(name="batch", bufs=2) as bpool,
            tc.tile_pool(name="ps_a", bufs=2, space="PSUM") as ps_a,
            tc.tile_pool(name="ps_g", bufs=2, space="PSUM") as ps_g,
            tc.tile_pool(name="ps_y2", bufs=2, space="PSUM") as ps_y2,
            tc.tile_pool(name="ps_wf", bufs=2, space="PSUM") as ps_wf,
        ):
            # ---------------- input DMAs ----------------
            # Sync HWDGE ring: mask (gates the scalar chain), then x, w0.
            mi = kpool.tile([1, BPC * T], I32)
            for b in range(BPC):
                nc.sync.dma_start(mi[0:1, b * T:(b + 1) * T], mk_d[b:b + 1, :])
            X2 = kpool.tile([128, BPC * N], F32R)  # [t, (b n)]
            nc.sync.dma_start(X2[:].rearrange("p (b n) -> p b n", b=BPC),
                              x_d[:].rearrange("b t n -> t b n"))
            w0sb = kpool.tile([128, BPC * 2 * N], F32R)  # [p, (b c m)]
            nc.sync.dma_start(w0sb[:].rearrange("p (b c m) -> p b c m", b=BPC, c=2),
                              w0_d[:].rearrange("b (c p) m -> p b c m", p=128))
            # ACT HWDGE ring: constants + la/eta (parallel with Sync's queue)
            identr = cpool.tile([128, 128], F32R)
            nc.scalar.dma_start(identr[:], cir_d[:])
            mko = cpool.tile([128, 256], F32)  # [:,0:128]=mk, row0 of [:,128:256]=ones
            nc.scalar.dma_start(mko[:], cmo_d[:])
            le_row = cpool.tile([1, 2 * H], F32)
            nc.scalar.dma_start(le_row[:], le_d[:])
            mk = mko[:, 0:128]
            ones1 = mko[0:1, 128:256]

            # pe8: la/eta replicated across partitions (cols 0..3 la, 4..7 eta)
            pe8_ps = ps_a.tile([128, 2 * H], F32, tag="a")
            nc.tensor.matmul(pe8_ps[:], ones1, le_row[:], start=True, stop=True)
            pe8 = cpool.tile([128, 2 * H], F32)
            nc.scalar.copy(pe8[:], pe8_ps[:])

            # ---------------- mask scalar chain (partition 0) ----------------
            # per-batch segments: [o:o+T]=mask(f32), [+T]=cm, [+2T]=cmx
            rows = kpool.tile([1, BPC * 3 * T], F32)
            for b in range(BPC):
                o = b * 3 * T
                nc.vector.tensor_copy(rows[0:1, o:o + T], mi[0:1, b * T:(b + 1) * T])
                nc.vector.tensor_tensor_scan(
                    rows[0:1, o + T:o + 2 * T], rows[0:1, o:o + T],
                    rows[0:1, o:o + T], 0.0, OP.add, OP.bypass)
                nc.vector.tensor_tensor(
                    rows[0:1, o + 2 * T:o + 3 * T], rows[0:1, o + T:o + 2 * T],
                    rows[0:1, o:o + T], OP.subtract)

            ODx2 = kpool.tile([128, BPC * EW], F32)
            E42 = kpool.tile([128, H * BPC * EW], F32)
            eb = lambda h, b: (h * BPC + b) * EW

            colsb, mEta, XTs, w0Ts, Gs, Y1s = [], [], [], [], [], []
            # ---------------- phase A per batch: scalars + shared matmuls ----
            for b in range(BPC):
                o = b * 3 * T

                # cols: [cm, cmx, cmT, ccol] on 128 partitions
                cols_ps = ps_a.tile([128, 2], F32, tag="a")
                for j in range(2):
                    nc.tensor.matmul(cols_ps[:, j:j + 1],
                                     rows[0:1, o + (j + 1) * T:o + (j + 2) * T],
                                     ones1[0:1, 0:1], start=True, stop=True)
                cols_b = bpool.tile([128, 4], F32)
                nc.scalar.copy(cols_b[:, 0:2], cols_ps[:])
                # bcc: col0 = cmT broadcast, cols 1..128 = cmx broadcast
                bcc_ps = ps_g.tile([128, 129], F32, tag="g")
                nc.tensor.matmul(bcc_ps[:], ones1,
                                 rows[0:1, o + 2 * T - 1:o + 3 * T],
                                 start=True, stop=True)
                nc.scalar.copy(cols_b[:, 2:3], bcc_ps[:, 0:1])
                nc.vector.scalar_tensor_tensor(
                    ODx2[:, b * EW:b * EW + 128], bcc_ps[:, 1:129],
                    cols_b[:, 0:1], mk, OP.subtract, OP.add)
                nc.vector.tensor_tensor(cols_b[:, 3:4], cols_b[:, 2:3],
                                        cols_b[:, 0:1], OP.subtract)
                nc.vector.tensor_copy(ODx2[:, b * EW + 128:b * EW + 131],
                                      cols_b[:, 1:4])
                colsb.append(cols_b)

                # mEta4 = (cm - cmx) * eta = m * eta, per head
                mEta4 = bpool.tile([128, H], F32)
                nc.vector.scalar_tensor_tensor(
                    mEta4[:], cols_b[:, 0:1].broadcast_to([128, H]),
                    cols_b[:, 1:2], pe8[:, H:2 * H], OP.subtract, OP.mult)
                mEta.append(mEta4)

                # transposes
                xt_ps = ps_a.tile([128, 256], F32R, tag="a")
                for d in range(2):
                    nc.tensor.transpose(
                        xt_ps[:, d * 128:(d + 1) * 128],
                        X2[:, b * N + d * 128: b * N + (d + 1) * 128], identr[:])
                XTsb = bpool.tile([128, 256], F32R)
                nc.scalar.copy(XTsb[:], xt_ps[:])
                XTs.append(XTsb)

                w0t_ps = ps_a.tile([128, 512], F32R, tag="a")
                for c in range(2):
                    for d in range(2):
                        nc.tensor.transpose(
                            w0t_ps[:, d * 256 + c * 128: d * 256 + (c + 1) * 128],
                            w0sb[:, (b * 2 + c) * N + d * 128: (b * 2 + c) * N + (d + 1) * 128],
                            identr[:])
                w0Tsb = bpool.tile([128, 512], F32R)
                nc.scalar.copy(w0Tsb[:], w0t_ps[:])
                w0Ts.append(w0Tsb)

                # G = X X^T  (PSUM, [u,t]) — consumed by AgT STTs in phase B
                g_ps = ps_g.tile([128, 128], F32, tag="g")
                for d in range(2):
                    nc.tensor.matmul(
                        g_ps[:], XTsb[:, d * 128:(d + 1) * 128],
                        XTsb[:, d * 128:(d + 1) * 128],
                        start=(d == 0), stop=(d == 1))
                Gs.append(g_ps)

                # Y1 = X @ w0^T
                y1_ps = ps_a.tile([128, 256], F32, tag="a")
                for d in range(2):
                    nc.tensor.matmul(
                        y1_ps[:], XTsb[:, d * 128:(d + 1) * 128],
                        w0Tsb[:, d * 256:(d + 1) * 256],
                        start=(d == 0), stop=(d == 1))
                Y1sb = bpool.tile([128, 256], F32)
                nc.scalar.copy(Y1sb[:], y1_ps[:])
                Y1s.append(Y1sb)

            # ---------------- exponentials: one op per head, both batches ----
            for h in range(H):
                nc.scalar.activation(
                    E42[:, h * BPC * EW:(h + 1) * BPC * EW], ODx2[:], AF.Exp,
                    scale=pe8[:, h:h + 1])

            # ---------------- phase B per batch: per-head outputs ----------
            for b in range(BPC):
                xb = X2[:, b * N:(b + 1) * N]
                cols_b, mEta4, g_ps, Y1sb = colsb[b], mEta[b], Gs[b], Y1s[b]

                # coef4 = ec * mEta
                coef4 = bpool.tile([128, H], F32)
                nc.vector.tensor_tensor(
                    coef4[:].rearrange("p (h c) -> p h c", c=1),
                    E42[:].rearrange("p (h c) -> p h c", c=BPC * EW)[
                        :, :, b * EW + 130:b * EW + 131],
                    mEta4[:].rearrange("p (h c) -> p h c", c=1), OP.mult)

                # AgT_h = (G * mEta_h) * E_h   (m*eta folded in)
                AgT4 = bpool.tile([128, H * 128], F32R)
                for h in range(H):
                    nc.vector.scalar_tensor_tensor(
                        AgT4[:, h * 128:(h + 1) * 128], g_ps[:],
                        mEta4[:, h:h + 1], E42[:, eb(h, b):eb(h, b) + 128],
                        OP.mult, OP.mult)

                # Xc_h = coef_h . X  (ACT per-partition scale)
                Xc4 = bpool.tile([128, H * N], F32R)
                for h in range(H):
                    nc.scalar.mul(Xc4[:, h * N:(h + 1) * N], xb,
                                  coef4[:, h:h + 1])

                Ysb = bpool.tile([128, H * N], F32)
                wfsb = bpool.tile([128, H * 2 * N], F32)
                for h in range(H):
                    # Y2 = AgT_h^T @ X -> PSUM ; Y = Y1*P + Y2
                    y2_ps = ps_y2.tile([128, 256], F32, tag="y2")
                    nc.tensor.matmul(
                        y2_ps[:], AgT4[:, h * 128:(h + 1) * 128], xb,
                        start=True, stop=True)
                    nc.vector.scalar_tensor_tensor(
                        Ysb[:, h * N:(h + 1) * N], Y1sb[:],
                        E42[:, eb(h, b) + 128:eb(h, b) + 129], y2_ps[:],
                        OP.mult, OP.add)

                    # wf: Sigma on PE; w0-term fused into the PSUM->SBUF move
                    wf_ps = ps_wf.tile([128, 512], F32, tag="wf")
                    for c in range(2):
                        nc.tensor.matmul(
                            wf_ps[:, c * 256:(c + 1) * 256],
                            Xc4[:, h * N + c * 128: h * N + (c + 1) * 128],
                            xb, start=True, stop=True)
                    nc.vector.scalar_tensor_tensor(
                        wfsb[:, h * 512:(h + 1) * 512],
                        w0sb[:, (b * 2) * N:(b * 2 + 2) * N],
                        E42[:, eb(h, b) + 129:eb(h, b) + 130], wf_ps[:],
                        OP.mult, OP.add)

                # outputs: y on ACT ring, wf (per head-pair) on Sync ring
                nc.scalar.dma_start(
                    y_d[:].rearrange("b t h n -> b t h n")[b],
                    Ysb[:].rearrange("p (h n) -> p h n", h=H))
                wf_view = wf_d[:].rearrange("b h (c p) m -> b p h c m", p=128)[b]
                for hp in range(2):
                    nc.sync.dma_start(
                        wf_view[:, hp * 2:(hp + 1) * 2],
                        wfsb[:, hp * 1024:(hp + 1) * 1024].rearrange(
                            "p (h c m) -> p h c m", h=2, c=2))

    nc.finalize()
    return nc


_NC_CACHE = None


def _get_nc():
    global _NC_CACHE
    if _NC_CACHE is None:
        _NC_CACHE = build_nc()
    return _NC_CACHE


def _consts():
    identr = np.eye(128, dtype=np.float32)
    mko = np.zeros((128, 256), np.float32)
    # mk[u,t] = 1e9 where t <= u (pre-exp causal mask)
    u = np.arange(128)
    mko[:, 0:128] = np.where(u[None, :] <= u[:, None], np.float32(1e9), np.float32(0))
    mko[0, 128:256] = 1.0  # ones row for K=1 broadcast matmuls
    return identr, mko


def make_in_maps(x, w_init, mask, alpha, eta):
    la = np.minimum(
        np.log(np.maximum(np.asarray(alpha, np.float64), 1e-30)), -1e-7
    ).astype(np.float32)
    laeta = np.concatenate([la, np.asarray(eta, np.float32)]).reshape(1, 2 * H)
    identr, mko = _consts()
    maps = []
    for c in range(NCORES):
        sl = slice(BPC * c, BPC * (c + 1))
        maps.append({
            "x_sh": np.ascontiguousarray(np.asarray(x, np.float32)[sl]),
            "w0_sh": np.ascontiguousarray(np.asarray(w_init, np.float32)[sl]),
            "mask_sh": np.ascontiguousarray(np.asarray(mask, np.int32)[sl]),
            "laeta": laeta,
            "c_identr": identr,
            "c_mko": mko,
        })
    return maps


def kernel(x, w_init, alpha, eta, mask):
    nc = _get_nc()
    in_maps = make_in_maps(x, w_init, mask, alpha, eta)
    res = run_bass_kernel_spmd(nc, in_maps, list(range(NCORES)))
    wf = np.concatenate([m["wf_out"] for m in res.results], axis=0)
    y = np.concatenate([m["y_out"] for m in res.results], axis=0)
    return wf, y
